# revision 24
# baseline (speedup 1.0000x reference)
"""Block-sparse attention kernel for Trainium2 (8 NeuronCores).

Problem: B=2, S=2048, H=16, Dqk=Dv=64, 64x64 block mask (30% + forced diag),
AND causal. out = softmax(mask(QK^T/8)) @ V.

Strategy
--------
- Shard the 32 (batch, head) pairs across 8 cores, 4 heads per core.
- Each core gets its OWN Bass program with the sparse block schedule baked in
  from its heads' block masks (compiled at call time, run concurrently on the
  8 axon devices).
- Per head, scores are computed TRANSPOSED (S^T[k, q]) so that P^T = exp(S^T)
  lands in SBUF in exactly the layout PV needs (k on partitions) — no on-chip
  transposes anywhere:
    * host supplies Q^T and K^T as [96(d+ind), 2048(s)] fp16, V as [128,
      16*65] fp16 "v-pair" tiles [V[2t]; V[2t+1]] with a ones column (col 64).
    * rows 64:96 of Q^T are 32 q-block indicator rows; rows 64:96 of K^T hold
      -30000 where block (qb=j, kb) is masked off (block-causal AND sparse
      mask): the QK matmul contracts 96 rows and lands masked scores directly,
      so exp() gives exact zeros for dead 64x64 half-blocks — no memsets.
    * k-blocks are processed in pairs (2t, 2t+1) = 128 partitions.
    * QK: matmul(lhsT=K^T pair [96,128], rhs=Q^T qb-run [96,64n]) -> PSUM.
    * exp: one ACT op per ~1024 PSUM columns (scale=1/8 fused), fp16 out.
    * fixups (DVE): multiply causal triangle into diagonal blocks (the only
      sub-block-granularity mask), one [128,128] pattern op per pair.
    * PV: matmul(lhsT=[V|1] pair [128,65], rhs=P^T run) accumulating O^T[65,
      2048] in PSUM across k-pairs. The program-order-first PV per PSUM bank
      uses start=True (clears the bank's has_written bits; later PVs
      accumulate or overwrite-on-first-touch per element).
    * O^T (unnormalized, with row 64 = softmax denominator l) is copied to
      SBUF and DMA'd out; the host divides and transposes back.
- The per-chunk pipeline is software-pipelined one deep (emit QK(i+1) before
  PV(i)) so the PE never stalls waiting for ACT/DVE of the current chunk.
- A post-scheduling pass rewrites same-weights matmul chains to load weights
  once (LDWEIGHTS dominates PE time for short runs).
- Softmax uses no running max: inputs are N(0,1) so scores/8 stay in a range
  where exp() is safely finite in fp32 (exp(~7) ~ 1e3).
"""

import threading
from contextlib import ExitStack

import numpy as np

import concourse.bass as bass
import concourse.tile as tile
from concourse import mybir
from concourse.bass_utils import run_bass_kernel_spmd
from concourse.vector_clock import ScopedClock

# ----------------------------------------------------------------------------
# Workaround: the installed walrus rejects instructions with more than one
# sync wait. Tile's kernel-tail drain attaches every outstanding clock sem to
# one Drain instruction; split them one wait per Drain.
# ----------------------------------------------------------------------------


def _split_drain_and_barrier(self, tick_clock, wait_clock):
    nc = self.nc
    drain_inst = nc.sync.drain()
    wait_clock.add_sem_waits(
        drain_inst.ins, ScopedClock({None: tick_clock.global_clock})
    )
    si = drain_inst.ins.sync_info
    waits = list(si.on_wait) if si is not None else []
    if len(waits) > 1:
        drain_inst.ins.sync_info = mybir.SyncInfo(
            on_wait=waits[:1], on_update=list(si.on_update)
        )
        for w in waits[1:]:
            d2 = nc.sync.drain()
            d2.ins.sync_info = mybir.SyncInfo(on_wait=[w], on_update=[])
    nc.all_engine_barrier()
    popped = nc._tile_sem_poison_stack.pop()
    assert popped is self._sem_poison
    nc.clear_and_free_semaphores(list(self.sems.allocated().values()))
    nc.all_engine_barrier()


tile.TileContext._drain_and_barrier = _split_drain_and_barrier


def _dedup_ldweights(nc):
    """Bacc lowers every matmul to an explicit Ldweights + non-self-loading
    Matmult pair; short same-weights runs then reload the identical weights
    every matmul, and LDWEIGHTS (~weight_cols/1.2GHz each) dominates PE time.
    Drop an Ldweights when the previously loaded weights are identical,
    preserving its semaphore waits/updates on a PE NOP in its place.

    Runs after TileContext exit (Bacc.compile already moved matmul waits onto
    the Ldweights), BEFORE _split_multi_waits.
    """
    for fn in nc.m.functions:
        for bb in fn.blocks:
            out = []
            last_key = None
            changed = False
            for inst in bb.instructions:
                if isinstance(inst, mybir.InstLdweights):
                    w = inst.ins[0]
                    key = (
                        str(getattr(w, "memref", None)),
                        w.offset,
                        str(w.ap),
                        str(getattr(w, "dtype", None)),
                        inst.tile_position,
                        inst.perf_mode,
                        inst.is_transpose,
                    )
                    if key == last_key:
                        si = inst.sync_info
                        waits = list(si.on_wait) if si else []
                        ups = list(si.on_update) if si else []
                        if waits or ups:
                            out.append(
                                mybir.InstNoOp(
                                    name=nc.get_next_instruction_name(),
                                    engine=inst.engine,
                                    sync_info=mybir.SyncInfo(
                                        on_wait=waits, on_update=ups
                                    ),
                                    bass_nofuse=True,
                                )
                            )
                        changed = True
                        continue
                    last_key = key
                    out.append(inst)
                elif isinstance(inst, mybir.InstMatmult):
                    out.append(inst)  # non-self-loading; weights undisturbed
                elif isinstance(inst, (mybir.InstNoOp, mybir.InstEventSemaphore)):
                    out.append(inst)
                else:
                    if inst.engine == mybir.EngineType.PE:
                        last_key = None
                    out.append(inst)
            if changed:
                bb.instructions = out


def _split_multi_waits(nc):
    """Hoist extra sync waits onto same-engine NOPs (walrus: 1 wait/inst)."""
    for fn in nc.m.functions:
        for bb in fn.blocks:
            out = []
            changed = False
            for inst in bb.instructions:
                si = inst.sync_info
                if si is not None and len(si.on_wait) > 1:
                    waits = list(si.on_wait)
                    for w in waits[:-1]:
                        out.append(
                            mybir.InstNoOp(
                                name=nc.get_next_instruction_name(),
                                engine=inst.engine,
                                sync_info=mybir.SyncInfo(on_wait=[w], on_update=[]),
                                bass_nofuse=True,
                            )
                        )
                    inst.sync_info = mybir.SyncInfo(
                        on_wait=[waits[-1]], on_update=list(si.on_update)
                    )
                    changed = True
                out.append(inst)
            if changed:
                bb.instructions = out

# ---------------------------------------------------------------------------
# Problem constants (hardcoded per the task contract)
# ---------------------------------------------------------------------------
B, S, H, D = 2, 2048, 16, 64
NB = 32  # number of 64-wide blocks along S
N_CORES = 8
HPC = 4  # heads (flat b*H+h) per core
CHUNK = 24  # score col-blocks per PSUM chunk (24*64 = 1536 fp32 = 3 banks)
DA = 96  # augmented contraction: 64 d rows + 32 q-block indicator rows
NEGM = -30000.0  # fp16-safe "minus infinity" for masked score bias
F16 = mybir.dt.float16
F32 = mybir.dt.float32


def _head_schedule(mask, pairs, gap=2):
    """Columns of the S^T score layout for one head.

    mask: [32, 32] bool. Active block (qb, kb) requires qb >= kb (block-level
    causal) and mask[qb, kb]. pairs: 16 (kb1, kb2) k-block pairs; pair t forms
    the 128-partition tile [K[kb1]; K[kb2]].

    Emission order is qb-bank-group-major (g = qb//8), then pair-major, so all
    of an O^T bank's PV writes are contiguous and each (g, t) shares one QK
    weight tile.

    Interior qb-gaps of <= `gap` within a (g, t) sequence are bridged with
    fake columns (top=bot=False -> fully masked, exp gives 0) so QK/PV runs
    merge into fewer, larger matmuls.
    """
    cols = []
    for g in range(NB // 8):
        for t, (kb1, kb2) in enumerate(pairs):
            seq = []
            for qb in range(8 * g, 8 * (g + 1)):
                top = qb >= kb1 and bool(mask[qb, kb1])
                bot = qb >= kb2 and bool(mask[qb, kb2])
                if top or bot:
                    seq.append((qb, top, bot))
            ext = []
            for idx, (qb, top, bot) in enumerate(seq):
                if ext:
                    prev_qb = ext[-1][0]
                    if 1 < qb - prev_qb <= gap + 1:
                        for fqb in range(prev_qb + 1, qb):
                            ext.append((fqb, False, False))
                ext.append((qb, top, bot))
            for qb, top, bot in ext:
                cols.append(
                    {
                        "t": t,
                        "qb": qb,
                        "top": top,
                        "bot": bot,
                        "kb1": kb1,
                        "kb2": kb2,
                        "g": g,
                    }
                )
    return cols


def _is_diag_pair(c, nxt):
    """col c = (qb==kb1, top tri) directly followed by its partner col
    (qb==kb2==qb+1, bot tri) of the same pair -> one [128,128] pattern op."""
    return (
        c["qb"] == c["kb1"]
        and c["top"]
        and nxt is not None
        and nxt["t"] == c["t"]
        and nxt["qb"] == c["qb"] + 1
        and nxt["qb"] == nxt["kb2"]
        and nxt["bot"]
    )


def _runs(chunk, key_consecutive, bank_of, flags=None):
    """Split a chunk (list of (idx, col)) into affine matmul runs.

    key_consecutive(prev, cur) -> bool: can cur extend the run?
    bank_of(idx, col) -> int: PSUM bank id of the run target; run must stay in
      one bank.
    flags(col) -> hashable: must be uniform within a run (or None).
    """
    runs = []
    cur = []
    for item in chunk:
        if cur:
            _, pc = cur[-1]
            _, cc = item
            ok = (
                key_consecutive(pc, cc)
                and bank_of(*item) == bank_of(*cur[0])
                and (flags is None or flags(cc) == flags(pc))
            )
            if ok:
                cur.append(item)
                continue
            runs.append(cur)
        cur = [item]
    if cur:
        runs.append(cur)
    return runs


def _chunks_of(cols):
    """Cut cols into chunks of <= CHUNK, never splitting a diagonal pair."""
    chunks = []
    cur = []
    i = 0
    while i < len(cols):
        nxt = cols[i + 1] if i + 1 < len(cols) else None
        take = 2 if _is_diag_pair(cols[i], nxt) else 1
        if len(cur) + take > CHUNK:
            chunks.append(cur)
            cur = []
        cur.extend(cols[i : i + take])
        i += take
    if cur:
        chunks.append(cur)
    return chunks


def build_program(schedules):
    """Build the Bass program for one core.

    schedules: list of HPC dicts {"pairs": [(kb1, kb2)]*16, "cols": [...]}.
    """
    nc = bass.Bass()
    qt = nc.declare_dram_parameter("qt", [HPC, DA, S], F16, isOutput=False)
    kt = nc.declare_dram_parameter("kt", [HPC, DA, S], F16, isOutput=False)
    va = nc.declare_dram_parameter("va", [HPC, 128, 16 * 65], F16, isOutput=False)
    tri = nc.declare_dram_parameter("tri", [128, 64], F16, isOutput=False)
    pats = nc.declare_dram_parameter("pats", [128, 128], F16, isOutput=False)
    ot = nc.declare_dram_parameter("ot", [HPC, 65, S], F32, isOutput=True)

    with tile.TileContext(nc) as tc, ExitStack() as ctx:
        const = ctx.enter_context(tc.tile_pool(name="const", bufs=1))
        pts = ctx.enter_context(tc.tile_pool(name="pts", bufs=4))
        outp = ctx.enter_context(tc.tile_pool(name="outp", bufs=4))
        psS = ctx.enter_context(tc.tile_pool(name="psS", bufs=2, space="PSUM"))
        psO = ctx.enter_context(tc.tile_pool(name="psO", bufs=2, space="PSUM"))

        # All input DMAs share the sync hardware queue (other engines' queues
        # would park a completion-wait in front of their compute work).
        # Transfer order = need order: the tiny fixup constants, then head 0's
        # tensors, then the rest; the first fixup/PV stall otherwise.
        tri_t = const.tile([128, 64], F16, tag="tri")
        nc.sync.dma_start(out=tri_t[:], in_=tri[:])
        pats_t = const.tile([128, 128], F16, tag="pats")
        nc.sync.dma_start(out=pats_t[:], in_=pats[:])
        qts, kts, vas = [], [], []
        for s in range(HPC):
            qs = const.tile([DA, S], F16, tag=f"qt{s}")
            ks = const.tile([DA, S], F16, tag=f"kt{s}")
            vs = const.tile([128, 16 * 65], F16, tag=f"va{s}")
            if s == 0:
                # Head 0 paces the pipeline start: land the first-half columns
                # (pairs t<8 / qb-groups 0-1 — all chunk-0 ever touches)
                # before the rest so the first QK issues ~3us earlier.
                nc.sync.dma_start(out=ks[:, 0 : S // 2], in_=kt[s][:, 0 : S // 2])
                nc.sync.dma_start(out=qs[:, 0 : S // 2], in_=qt[s][:, 0 : S // 2])
                nc.sync.dma_start(out=vs[:], in_=va[s])
                nc.sync.dma_start(out=ks[:, S // 2 :], in_=kt[s][:, S // 2 :])
                nc.sync.dma_start(out=qs[:, S // 2 :], in_=qt[s][:, S // 2 :])
            else:
                nc.sync.dma_start(out=ks[:], in_=kt[s])
                nc.sync.dma_start(out=qs[:], in_=qt[s])
                nc.sync.dma_start(out=vs[:], in_=va[s])
            qts.append(qs)
            kts.append(ks)
            vas.append(vs)
        zeros = const.tile([128, 512], F16, tag="zeros")
        nc.vector.memset(zeros[:], 0.0)

        # PE warm-up: the HAM clock gate keeps a cold PE at 1.2 GHz; burn
        # ~4 us of dummy matmuls (overlapping the input DMAs) to reach 2.4.
        wps = psS.tile([128, 64 * CHUNK], F32, tag="ps")
        for _ in range(8):
            nc.tensor.matmul(
                wps[:, 0:512],
                lhsT=zeros[:, 0:128],
                rhs=zeros[:, 0:512],
                start=True,
                stop=True,
            )

        # One flat chunk stream across all heads so the software pipeline
        # (and the PE) never drains at head boundaries.
        stream = []  # (s, key=(s, ci), chunk_cols)
        last_chunk_of_group = {}  # (s, g) -> key of chunk with g's last col
        for s in range(HPC):
            for ci, chunk_cols in enumerate(_chunks_of(schedules[s]["cols"])):
                key = (s, ci)
                stream.append((s, key, chunk_cols))
                for col in chunk_cols:
                    last_chunk_of_group[(s, col["qb"] // 8)] = key

        oTs = {}  # (s, g) -> [tile, opened_flag]

        def get_oT(s_, g_):
            if (s_, g_) not in oTs:
                oTs[(s_, g_)] = [
                    psO.tile([128, 512], F32, name=f"oT{s_}_{g_}", tag="psO"),
                    False,
                ]
            return oTs[(s_, g_)]

        def close_group(s_, g_):
            t_, _ = oTs.pop((s_, g_))
            o_sb = outp.tile([65, 512], F32, name=f"osb{s_}_{g_}", tag="o")
            nc.vector.tensor_copy(out=o_sb[:], in_=t_[0:65, :])
            nc.sync.dma_start(
                out=ot[s_][:, 512 * g_ : 512 * (g_ + 1)], in_=o_sb[:]
            )

        def emit_qk(s_, chunk, ps):
            qk = _runs(
                chunk,
                key_consecutive=lambda p, c: p["t"] == c["t"]
                and c["qb"] == p["qb"] + 1,
                bank_of=lambda i, c: i // 8,
            )
            for run in qk:
                i0, rc = run[0]
                n = len(run)
                nc.tensor.matmul(
                    ps[:, 64 * i0 : 64 * (i0 + n)],
                    lhsT=kts[s_][:, 128 * rc["t"] : 128 * (rc["t"] + 1)],
                    rhs=qts[s_][:, 64 * rc["qb"] : 64 * (rc["qb"] + n)],
                    start=True,
                    stop=True,
                )

        def emit_fixups(chunk, pt):
            # Only sub-block mask left after the QK mask-fold: the causal
            # triangle on diagonal blocks. Adjacent diag pairs use one
            # [128,128] pattern op; stragglers use a [64,64] tri op.
            L = len(chunk)
            i = 0
            while i < L:
                c = chunk[i][1]
                if _is_diag_pair(c, chunk[i + 1][1] if i + 1 < L else None):
                    nc.vector.tensor_mul(
                        pt[:, 64 * i : 64 * (i + 2)],
                        pt[:, 64 * i : 64 * (i + 2)],
                        pats_t[:],
                    )
                    i += 2
                    continue
                if c["top"] and c["qb"] == c["kb1"]:
                    nc.vector.tensor_mul(
                        pt[0:64, 64 * i : 64 * (i + 1)],
                        pt[0:64, 64 * i : 64 * (i + 1)],
                        tri_t[0:64],
                    )
                if c["bot"] and c["qb"] == c["kb2"]:
                    nc.vector.tensor_mul(
                        pt[64:128, 64 * i : 64 * (i + 1)],
                        pt[64:128, 64 * i : 64 * (i + 1)],
                        tri_t[64:128],
                    )
                i += 1

        def emit_pv(s_, chunk, pt):
            pv = _runs(
                chunk,
                key_consecutive=lambda p, c: p["t"] == c["t"]
                and c["qb"] == p["qb"] + 1,
                bank_of=lambda i, c: c["qb"] // 8,
            )
            for run in pv:
                i0, rc = run[0]
                n = len(run)
                g_ = rc["qb"] // 8
                ent = get_oT(s_, g_)
                first = not ent[1]
                ent[1] = True
                q0 = rc["qb"] - 8 * g_
                nc.tensor.matmul(
                    ent[0][0:65, 64 * q0 : 64 * (q0 + n)],
                    lhsT=vas[s_][:, 65 * rc["t"] : 65 * (rc["t"] + 1)],
                    rhs=pt[:, 64 * i0 : 64 * (i0 + n)],
                    start=first,
                    stop=True,
                    skip_group_check=True,
                )

        # Software-pipelined chunk loop: PE order is QK(0), QK(1), PV(0),
        # QK(2), PV(1), ..., PV(last) so the PE works on the next chunk's
        # scores while ACT+DVE process the current one.
        pending = None  # (s, key, chunk, pt) awaiting PV emission
        for s, key, chunk_cols in stream:
            chunk = list(enumerate(chunk_cols))
            L = len(chunk)
            ps = psS.tile([128, 64 * CHUNK], F32, tag="ps")
            emit_qk(s, chunk, ps)
            if pending is not None:
                emit_pv(pending[0], pending[2], pending[3])
                for sg in [
                    sg2
                    for sg2, lc in last_chunk_of_group.items()
                    if lc == pending[1]
                ]:
                    close_group(*sg)
            pt = pts.tile([128, 64 * CHUNK], F16, tag="pt")
            nc.scalar.activation(
                out=pt[:, : 64 * L],
                in_=ps[:, : 64 * L],
                func=mybir.ActivationFunctionType.Exp,
                scale=0.125,
            )
            emit_fixups(chunk, pt)
            pending = (s, key, chunk, pt)
        emit_pv(pending[0], pending[2], pending[3])
        for sg in sorted(oTs):
            close_group(*sg)

    _dedup_ldweights(nc)
    _split_multi_waits(nc)
    return nc


def _prep_inputs(q, k, v, schedules):
    """Per-core input arrays keyed as the programs expect."""
    # flat head g = b*H + h
    qt_nat = q.transpose(0, 2, 3, 1).reshape(B * H, D, S).astype(np.float16)
    kt_nat = k.transpose(0, 2, 3, 1).reshape(B * H, D, S).astype(np.float16)
    kt_nat = kt_nat.reshape(B * H, D, NB, 64)
    # augmented Q^T: rows 64:96 are q-block indicators [qb(q) == j]
    qind = np.zeros((NB, S), np.float16)
    for j in range(NB):
        qind[j, 64 * j : 64 * (j + 1)] = 1.0
    qt_all = np.zeros((B * H, DA, S), np.float16)
    qt_all[:, :D, :] = qt_nat
    qt_all[:, D : D + NB, :] = qind[None]
    # augmented K^T: pair-ordered K rows + mask rows kt[64+j, kb-col] = NEGM
    # where block (qb=j, kb) is dead (block-causal AND sparse mask)
    masks_all = np.asarray(schedules[0]["masks_all"])
    kt_all = np.zeros((B * H, DA, S), np.float16)
    for g in range(B * H):
        order = [kb for p in schedules[g]["pairs"] for kb in p]
        kt_all[g, :D] = kt_nat[g][:, order, :].reshape(D, S)
        m = masks_all[g]  # [32 qb, 32 kb] bool, causal applied separately
        for pos, kb in enumerate(order):
            dead = np.ones(NB, np.float16) * NEGM
            for j in range(NB):
                if j >= kb and m[j, kb]:
                    dead[j] = 0.0
            kt_all[g, D : D + NB, 64 * pos : 64 * (pos + 1)] = dead[:, None]
    v_aug = np.concatenate([v, np.ones((B, S, H, 1), v.dtype)], axis=3)  # [B,S,H,65]
    vb_all = v_aug.transpose(0, 2, 1, 3).reshape(B * H, NB, 64, 65)  # [g, kb, 64, 65]
    # va[g]: per pair t, rows 0:64 = V[kb1] block, rows 64:128 = V[kb2]
    va_all = np.zeros((B * H, 128, 16 * 65), np.float16)
    for g in range(B * H):
        for t, (kb1, kb2) in enumerate(schedules[g]["pairs"]):
            va_all[g, 0:64, 65 * t : 65 * (t + 1)] = vb_all[g, kb1]
            va_all[g, 64:128, 65 * t : 65 * (t + 1)] = vb_all[g, kb2]
    # tri[kl, ql] = 1 where kl <= ql (allowed), both halves
    triu = np.triu(np.ones((64, 64), np.float16))
    tri_full = np.ascontiguousarray(np.concatenate([triu, triu], axis=0))
    # Diagonal-pair pattern [128, 128] for adjacent cols (qb=2t, qb=2t+1):
    # tri on the two diagonal sub-blocks, 1 elsewhere (dead halves are already
    # exact zeros from the QK mask-fold).
    one = np.ones((64, 64), np.float16)
    pat = np.block([[triu, one], [one, triu]]).astype(np.float16)
    pats_full = np.ascontiguousarray(pat)
    in_maps = []
    for c in range(N_CORES):
        sl = slice(HPC * c, HPC * (c + 1))
        in_maps.append(
            {
                "qt": np.ascontiguousarray(qt_all[sl]),
                "kt": np.ascontiguousarray(kt_all[sl]),
                "va": va_all[sl],
                "tri": tri_full,
                "pats": pats_full,
            }
        )
    return in_maps


def _match_pairs(mask, adj_bonus=1.5):
    """Pair k-blocks to maximize overlap of their active-q sets (greedy
    max-weight matching). Overlapping pairs make dual-dense score columns,
    shrinking the union column count that drives QK/exp/PV work. Adjacent
    pairs (i, i+1) get a bonus: their two diagonal-block triangle fixups
    merge into one [128,128] DVE op."""
    act = {
        kb: frozenset(qb for qb in range(kb, NB) if mask[qb, kb]) for kb in range(NB)
    }
    left = set(range(NB))
    pairs = []
    while left:
        best = None
        for i in left:
            for j in left:
                if j <= i:
                    continue
                sc = len(act[i] & act[j]) + (adj_bonus if j == i + 1 else 0.0)
                if best is None or sc > best[0] or (sc == best[0] and (i, j) < best[1:]):
                    best = (sc, i, j)
        _, i, j = best
        pairs.append((i, j))
        left -= {i, j}
    pairs.sort()
    return pairs


def _schedules(block_mask):
    """Per flat head: k-block pairing + column schedule."""
    masks_all = np.asarray(block_mask).reshape(B * H, NB, NB)
    scheds = []
    for g in range(B * H):
        pairs = [(2 * t, 2 * t + 1) for t in range(NB // 2)]
        scheds.append(
            {
                "pairs": pairs,
                "cols": _head_schedule(masks_all[g], pairs, gap=0),
                "masks_all": masks_all,
            }
        )
    return scheds


_PROG_CACHE = {}


def _get_programs(block_mask, schedules):
    key = np.asarray(block_mask).tobytes()
    if key not in _PROG_CACHE:
        _PROG_CACHE[key] = [
            build_program(schedules[HPC * c : HPC * (c + 1)]) for c in range(N_CORES)
        ]
    return _PROG_CACHE[key]


def run_cores(ncs, in_maps, trace=False):
    """Run the 8 per-core programs concurrently on the 8 devices."""
    import jax

    devs = jax.devices()
    results = [None] * N_CORES
    errs = [None] * N_CORES

    def _run(c):
        try:
            with jax.default_device(devs[c]):
                r = run_bass_kernel_spmd(
                    ncs[c], [in_maps[c]], core_ids=[0], trace=trace and c == 0
                )
                results[c] = r
        except Exception as e:  # noqa: BLE001
            errs[c] = e

    threads = [threading.Thread(target=_run, args=(c,)) for c in range(N_CORES)]
    for t in threads:
        t.start()
    for t in threads:
        t.join()
    for c, e in enumerate(errs):
        if e is not None:
            raise RuntimeError(f"core {c} failed") from e
    return results


def kernel(q, k, v, block_mask):
    q = np.asarray(q, dtype=np.float32)
    k = np.asarray(k, dtype=np.float32)
    v = np.asarray(v, dtype=np.float32)
    block_mask = np.asarray(block_mask).astype(bool)

    schedules = _schedules(block_mask)
    in_maps = _prep_inputs(q, k, v, schedules)
    ncs = _get_programs(block_mask, schedules)
    results = run_cores(ncs, in_maps)

    out = np.empty((B, S, H, D), np.float32)
    for c in range(N_CORES):
        ot = results[c].results[0]["ot"]  # [HPC, 65, S]
        for s in range(HPC):
            g = HPC * c + s
            b, h = divmod(g, H)
            o_un = ot[s, :D, :]  # [D, S] unnormalized
            l = ot[s, D, :]  # [S]
            out[b, :, h, :] = (o_un / l[None, :]).T
    return out


# revision 25
# speedup vs baseline: 1.2078x; 1.2078x over previous
"""Block-sparse attention kernel for Trainium2 (8 NeuronCores).

Problem: B=2, S=2048, H=16, Dqk=Dv=64, 64x64 block mask (30% + forced diag),
AND causal. out = softmax(mask(QK^T/8)) @ V.

Strategy
--------
- Shard the 32 (batch, head) pairs across 8 cores, 4 heads per core.
- Each core gets its OWN Bass program with the sparse block schedule baked in
  from its heads' block masks (compiled at call time, run concurrently on the
  8 axon devices).
- Per head, scores are computed TRANSPOSED (S^T[k, q]) so that P^T = exp(S^T)
  lands in SBUF in exactly the layout PV needs (k on partitions) — no on-chip
  transposes anywhere:
    * host supplies Q^T and K^T as [96(d+ind), 2048(s)] fp16, V as [128,
      16*65] fp16 "v-pair" tiles [V[2t]; V[2t+1]] with a ones column (col 64).
    * rows 64:96 of Q^T are 32 q-block indicator rows; rows 64:96 of K^T hold
      -30000 where block (qb=j, kb) is masked off (block-causal AND sparse
      mask): the QK matmul contracts 96 rows and lands masked scores directly,
      so exp() gives exact zeros for dead 64x64 half-blocks — no memsets.
    * k-blocks are processed in pairs (2t, 2t+1) = 128 partitions.
    * QK: matmul(lhsT=K^T pair [96,128], rhs=Q^T qb-run [96,64n]) -> PSUM.
    * exp: one ACT op per ~1024 PSUM columns (scale=1/8 fused), fp16 out.
    * fixups (DVE): multiply causal triangle into diagonal blocks (the only
      sub-block-granularity mask), one [128,128] pattern op per pair.
    * PV: matmul(lhsT=[V|1] pair [128,65], rhs=P^T run) accumulating O^T[65,
      2048] in PSUM across k-pairs. The program-order-first PV per PSUM bank
      uses start=True (clears the bank's has_written bits; later PVs
      accumulate or overwrite-on-first-touch per element).
    * O^T (unnormalized, with row 64 = softmax denominator l) is copied to
      SBUF and DMA'd out; the host divides and transposes back.
- The per-chunk pipeline is software-pipelined one deep (emit QK(i+1) before
  PV(i)) so the PE never stalls waiting for ACT/DVE of the current chunk.
- A post-scheduling pass rewrites same-weights matmul chains to load weights
  once (LDWEIGHTS dominates PE time for short runs).
- Softmax uses no running max: inputs are N(0,1) so scores/8 stay in a range
  where exp() is safely finite in fp32 (exp(~7) ~ 1e3).
"""

import threading
from contextlib import ExitStack

import numpy as np

import concourse.bass as bass
import concourse.tile as tile
from concourse import mybir
from concourse.bass_utils import run_bass_kernel_spmd
from concourse.vector_clock import ScopedClock

# ----------------------------------------------------------------------------
# Workaround: the installed walrus rejects instructions with more than one
# sync wait. Tile's kernel-tail drain attaches every outstanding clock sem to
# one Drain instruction; split them one wait per Drain.
# ----------------------------------------------------------------------------


def _split_drain_and_barrier(self, tick_clock, wait_clock):
    nc = self.nc
    drain_inst = nc.sync.drain()
    wait_clock.add_sem_waits(
        drain_inst.ins, ScopedClock({None: tick_clock.global_clock})
    )
    si = drain_inst.ins.sync_info
    waits = list(si.on_wait) if si is not None else []
    if len(waits) > 1:
        drain_inst.ins.sync_info = mybir.SyncInfo(
            on_wait=waits[:1], on_update=list(si.on_update)
        )
        for w in waits[1:]:
            d2 = nc.sync.drain()
            d2.ins.sync_info = mybir.SyncInfo(on_wait=[w], on_update=[])
    nc.all_engine_barrier()
    popped = nc._tile_sem_poison_stack.pop()
    assert popped is self._sem_poison
    nc.clear_and_free_semaphores(list(self.sems.allocated().values()))
    nc.all_engine_barrier()


tile.TileContext._drain_and_barrier = _split_drain_and_barrier


def _dedup_ldweights(nc):
    """Bacc lowers every matmul to an explicit Ldweights + non-self-loading
    Matmult pair; short same-weights runs then reload the identical weights
    every matmul, and LDWEIGHTS (~weight_cols/1.2GHz each) dominates PE time.
    Drop an Ldweights when the previously loaded weights are identical,
    preserving its semaphore waits/updates on a PE NOP in its place.

    Runs after TileContext exit (Bacc.compile already moved matmul waits onto
    the Ldweights), BEFORE _split_multi_waits.
    """
    for fn in nc.m.functions:
        for bb in fn.blocks:
            out = []
            last_key = None
            changed = False
            for inst in bb.instructions:
                if isinstance(inst, mybir.InstLdweights):
                    w = inst.ins[0]
                    key = (
                        str(getattr(w, "memref", None)),
                        w.offset,
                        str(w.ap),
                        str(getattr(w, "dtype", None)),
                        inst.tile_position,
                        inst.perf_mode,
                        inst.is_transpose,
                    )
                    if key == last_key:
                        si = inst.sync_info
                        waits = list(si.on_wait) if si else []
                        ups = list(si.on_update) if si else []
                        if waits or ups:
                            out.append(
                                mybir.InstNoOp(
                                    name=nc.get_next_instruction_name(),
                                    engine=inst.engine,
                                    sync_info=mybir.SyncInfo(
                                        on_wait=waits, on_update=ups
                                    ),
                                    bass_nofuse=True,
                                )
                            )
                        changed = True
                        continue
                    last_key = key
                    out.append(inst)
                elif isinstance(inst, mybir.InstMatmult):
                    out.append(inst)  # non-self-loading; weights undisturbed
                elif isinstance(inst, (mybir.InstNoOp, mybir.InstEventSemaphore)):
                    out.append(inst)
                else:
                    if inst.engine == mybir.EngineType.PE:
                        last_key = None
                    out.append(inst)
            if changed:
                bb.instructions = out


def _split_multi_waits(nc):
    """Hoist extra sync waits onto same-engine NOPs (walrus: 1 wait/inst)."""
    for fn in nc.m.functions:
        for bb in fn.blocks:
            out = []
            changed = False
            for inst in bb.instructions:
                si = inst.sync_info
                if si is not None and len(si.on_wait) > 1:
                    waits = list(si.on_wait)
                    for w in waits[:-1]:
                        out.append(
                            mybir.InstNoOp(
                                name=nc.get_next_instruction_name(),
                                engine=inst.engine,
                                sync_info=mybir.SyncInfo(on_wait=[w], on_update=[]),
                                bass_nofuse=True,
                            )
                        )
                    inst.sync_info = mybir.SyncInfo(
                        on_wait=[waits[-1]], on_update=list(si.on_update)
                    )
                    changed = True
                out.append(inst)
            if changed:
                bb.instructions = out

# ---------------------------------------------------------------------------
# Problem constants (hardcoded per the task contract)
# ---------------------------------------------------------------------------
B, S, H, D = 2, 2048, 16, 64
NB = 32  # number of 64-wide blocks along S
N_CORES = 8
HPC = 4  # heads (flat b*H+h) per core
CHUNK = 24  # score col-blocks per PSUM chunk (24*64 = 1536 fp32 = 3 banks)
DA = 96  # augmented contraction: 64 d rows + 32 q-block indicator rows
NEGM = -30000.0  # fp16-safe "minus infinity" for masked score bias
F16 = mybir.dt.float16
F32 = mybir.dt.float32


def _head_schedule(mask, pairs, gap=2):
    """Columns of the S^T score layout for one head.

    mask: [32, 32] bool. Active block (qb, kb) requires qb >= kb (block-level
    causal) and mask[qb, kb]. pairs: 16 (kb1, kb2) k-block pairs; pair t forms
    the 128-partition tile [K[kb1]; K[kb2]].

    Emission order is qb-bank-group-major (g = qb//8), then pair-major, so all
    of an O^T bank's PV writes are contiguous and each (g, t) shares one QK
    weight tile.

    Interior qb-gaps of <= `gap` within a (g, t) sequence are bridged with
    fake columns (top=bot=False -> fully masked, exp gives 0) so QK/PV runs
    merge into fewer, larger matmuls.
    """
    cols = []
    for g in range(NB // 8):
        for t, (kb1, kb2) in enumerate(pairs):
            seq = []
            for qb in range(8 * g, 8 * (g + 1)):
                top = qb >= kb1 and bool(mask[qb, kb1])
                bot = qb >= kb2 and bool(mask[qb, kb2])
                if top or bot:
                    seq.append((qb, top, bot))
            ext = []
            for idx, (qb, top, bot) in enumerate(seq):
                if ext:
                    prev_qb = ext[-1][0]
                    if 1 < qb - prev_qb <= gap + 1:
                        for fqb in range(prev_qb + 1, qb):
                            ext.append((fqb, False, False))
                ext.append((qb, top, bot))
            for qb, top, bot in ext:
                cols.append(
                    {
                        "t": t,
                        "qb": qb,
                        "top": top,
                        "bot": bot,
                        "kb1": kb1,
                        "kb2": kb2,
                        "g": g,
                    }
                )
    return cols


def _is_diag_pair(c, nxt):
    """col c = (qb==kb1, top tri) directly followed by its partner col
    (qb==kb2==qb+1, bot tri) of the same pair -> one [128,128] pattern op."""
    return (
        c["qb"] == c["kb1"]
        and c["top"]
        and nxt is not None
        and nxt["t"] == c["t"]
        and nxt["qb"] == c["qb"] + 1
        and nxt["qb"] == nxt["kb2"]
        and nxt["bot"]
    )


def _runs(chunk, key_consecutive, bank_of, flags=None):
    """Split a chunk (list of (idx, col)) into affine matmul runs.

    key_consecutive(prev, cur) -> bool: can cur extend the run?
    bank_of(idx, col) -> int: PSUM bank id of the run target; run must stay in
      one bank.
    flags(col) -> hashable: must be uniform within a run (or None).
    """
    runs = []
    cur = []
    for item in chunk:
        if cur:
            _, pc = cur[-1]
            _, cc = item
            ok = (
                key_consecutive(pc, cc)
                and bank_of(*item) == bank_of(*cur[0])
                and (flags is None or flags(cc) == flags(pc))
            )
            if ok:
                cur.append(item)
                continue
            runs.append(cur)
        cur = [item]
    if cur:
        runs.append(cur)
    return runs


def _chunks_of(cols):
    """Cut cols into chunks of <= CHUNK, never splitting a diagonal pair."""
    chunks = []
    cur = []
    i = 0
    while i < len(cols):
        nxt = cols[i + 1] if i + 1 < len(cols) else None
        take = 2 if _is_diag_pair(cols[i], nxt) else 1
        if len(cur) + take > CHUNK:
            chunks.append(cur)
            cur = []
        cur.extend(cols[i : i + take])
        i += take
    if cur:
        chunks.append(cur)
    return chunks


def build_program(schedules):
    """Build the Bass program for one core.

    schedules: list of HPC dicts {"pairs": [(kb1, kb2)]*16, "cols": [...]}.
    """
    nc = bass.Bass()
    qt = nc.declare_dram_parameter("qt", [HPC, DA, S], F16, isOutput=False)
    kt = nc.declare_dram_parameter("kt", [HPC, DA, S], F16, isOutput=False)
    va = nc.declare_dram_parameter("va", [HPC, 128, 16 * 65], F16, isOutput=False)
    tri = nc.declare_dram_parameter("tri", [128, 64], F16, isOutput=False)
    pats = nc.declare_dram_parameter("pats", [128, 128], F16, isOutput=False)
    ot = nc.declare_dram_parameter("ot", [HPC, 65, S], F32, isOutput=True)

    with tile.TileContext(nc) as tc, ExitStack() as ctx:
        const = ctx.enter_context(tc.tile_pool(name="const", bufs=1))
        pts = ctx.enter_context(tc.tile_pool(name="pts", bufs=3))
        outp = ctx.enter_context(tc.tile_pool(name="outp", bufs=3))
        psS = ctx.enter_context(tc.tile_pool(name="psS", bufs=2, space="PSUM"))
        psO = ctx.enter_context(tc.tile_pool(name="psO", bufs=2, space="PSUM"))

        # All input DMAs share the sync hardware queue (other engines' queues
        # would park a completion-wait in front of their compute work).
        # Transfer order = need order: the tiny fixup constants, then head 0's
        # tensors, then the rest; the first fixup/PV stall otherwise.
        tri_t = const.tile([128, 64], F16, tag="tri")
        nc.sync.dma_start(out=tri_t[:], in_=tri[:])
        pats_t = const.tile([128, 128], F16, tag="pats")
        nc.sync.dma_start(out=pats_t[:], in_=pats[:])
        qts, kts, vas = [], [], []
        for s in range(HPC):
            qs = const.tile([DA, S], F16, tag=f"qt{s}")
            ks = const.tile([DA, S], F16, tag=f"kt{s}")
            vs = const.tile([128, 16 * 65], F16, tag=f"va{s}")
            if s == 0:
                # Head 0 paces the pipeline start: land the first-half columns
                # (pairs t<8 / qb-groups 0-1 — all chunk-0 ever touches)
                # before the rest so the first QK issues ~3us earlier.
                nc.sync.dma_start(out=ks[:, 0 : S // 2], in_=kt[s][:, 0 : S // 2])
                nc.sync.dma_start(out=qs[:, 0 : S // 2], in_=qt[s][:, 0 : S // 2])
                nc.sync.dma_start(out=vs[:], in_=va[s])
                nc.sync.dma_start(out=ks[:, S // 2 :], in_=kt[s][:, S // 2 :])
                nc.sync.dma_start(out=qs[:, S // 2 :], in_=qt[s][:, S // 2 :])
            else:
                nc.sync.dma_start(out=ks[:], in_=kt[s])
                nc.sync.dma_start(out=qs[:], in_=qt[s])
                nc.sync.dma_start(out=vs[:], in_=va[s])
            qts.append(qs)
            kts.append(ks)
            vas.append(vs)
        zeros = const.tile([128, 512], F16, tag="zeros")
        nc.vector.memset(zeros[:], 0.0)

        # PE warm-up: the HAM clock gate keeps a cold PE at 1.2 GHz; burn
        # ~4 us of dummy matmuls (overlapping the input DMAs) to reach 2.4.
        wps = psS.tile([128, 64 * CHUNK], F32, tag="ps")
        for _ in range(8):
            nc.tensor.matmul(
                wps[:, 0:512],
                lhsT=zeros[:, 0:128],
                rhs=zeros[:, 0:512],
                start=True,
                stop=True,
            )

        # One flat chunk stream across all heads so the software pipeline
        # (and the PE) never drains at head boundaries.
        stream = []  # (s, key=(s, ci), chunk_cols)
        last_chunk_of_group = {}  # (s, g) -> key of chunk with g's last col
        for s in range(HPC):
            for ci, chunk_cols in enumerate(_chunks_of(schedules[s]["cols"])):
                key = (s, ci)
                stream.append((s, key, chunk_cols))
                for col in chunk_cols:
                    last_chunk_of_group[(s, col["qb"] // 8)] = key

        oTs = {}  # (s, g) -> [tile, opened_flag]

        def get_oT(s_, g_):
            if (s_, g_) not in oTs:
                oTs[(s_, g_)] = [
                    psO.tile([128, 512], F32, name=f"oT{s_}_{g_}", tag="psO"),
                    False,
                ]
            return oTs[(s_, g_)]

        def close_group(s_, g_):
            t_, _ = oTs.pop((s_, g_))
            o_sb = outp.tile([65, 512], F32, name=f"osb{s_}_{g_}", tag="o")
            nc.vector.tensor_copy(out=o_sb[:], in_=t_[0:65, :])
            nc.sync.dma_start(
                out=ot[s_][:, 512 * g_ : 512 * (g_ + 1)], in_=o_sb[:]
            )

        def emit_qk(s_, chunk, ps):
            qk = _runs(
                chunk,
                key_consecutive=lambda p, c: p["t"] == c["t"]
                and c["qb"] == p["qb"] + 1,
                bank_of=lambda i, c: i // 8,
            )
            for run in qk:
                i0, rc = run[0]
                n = len(run)
                nc.tensor.matmul(
                    ps[:, 64 * i0 : 64 * (i0 + n)],
                    lhsT=kts[s_][:, 128 * rc["t"] : 128 * (rc["t"] + 1)],
                    rhs=qts[s_][:, 64 * rc["qb"] : 64 * (rc["qb"] + n)],
                    start=True,
                    stop=True,
                )

        def emit_fixups(chunk, pt):
            # Only sub-block mask left after the QK mask-fold: the causal
            # triangle on diagonal blocks. Adjacent diag pairs use one
            # [128,128] pattern op; stragglers use a [64,64] tri op.
            L = len(chunk)
            i = 0
            while i < L:
                c = chunk[i][1]
                if _is_diag_pair(c, chunk[i + 1][1] if i + 1 < L else None):
                    nc.vector.tensor_mul(
                        pt[:, 64 * i : 64 * (i + 2)],
                        pt[:, 64 * i : 64 * (i + 2)],
                        pats_t[:],
                    )
                    i += 2
                    continue
                if c["top"] and c["qb"] == c["kb1"]:
                    nc.vector.tensor_mul(
                        pt[0:64, 64 * i : 64 * (i + 1)],
                        pt[0:64, 64 * i : 64 * (i + 1)],
                        tri_t[0:64],
                    )
                if c["bot"] and c["qb"] == c["kb2"]:
                    nc.vector.tensor_mul(
                        pt[64:128, 64 * i : 64 * (i + 1)],
                        pt[64:128, 64 * i : 64 * (i + 1)],
                        tri_t[64:128],
                    )
                i += 1

        def emit_pv(s_, chunk, pt):
            pv = _runs(
                chunk,
                key_consecutive=lambda p, c: p["t"] == c["t"]
                and c["qb"] == p["qb"] + 1,
                bank_of=lambda i, c: c["qb"] // 8,
            )
            for run in pv:
                i0, rc = run[0]
                n = len(run)
                g_ = rc["qb"] // 8
                ent = get_oT(s_, g_)
                first = not ent[1]
                ent[1] = True
                q0 = rc["qb"] - 8 * g_
                nc.tensor.matmul(
                    ent[0][0:65, 64 * q0 : 64 * (q0 + n)],
                    lhsT=vas[s_][:, 65 * rc["t"] : 65 * (rc["t"] + 1)],
                    rhs=pt[:, 64 * i0 : 64 * (i0 + n)],
                    start=first,
                    stop=True,
                    skip_group_check=True,
                )

        # Software-pipelined chunk loop: PE order is QK(0), QK(1), PV(0),
        # QK(2), PV(1), ..., PV(last) so the PE works on the next chunk's
        # scores while ACT+DVE process the current one.
        pending = None  # (s, key, chunk, pt) awaiting PV emission
        for s, key, chunk_cols in stream:
            chunk = list(enumerate(chunk_cols))
            L = len(chunk)
            ps = psS.tile([128, 64 * CHUNK], F32, tag="ps")
            emit_qk(s, chunk, ps)
            if pending is not None:
                emit_pv(pending[0], pending[2], pending[3])
                for sg in [
                    sg2
                    for sg2, lc in last_chunk_of_group.items()
                    if lc == pending[1]
                ]:
                    close_group(*sg)
            pt = pts.tile([128, 64 * CHUNK], F16, tag="pt")
            nc.scalar.activation(
                out=pt[:, : 64 * L],
                in_=ps[:, : 64 * L],
                func=mybir.ActivationFunctionType.Exp,
                scale=0.125,
            )
            emit_fixups(chunk, pt)
            pending = (s, key, chunk, pt)
        emit_pv(pending[0], pending[2], pending[3])
        for sg in sorted(oTs):
            close_group(*sg)

    _dedup_ldweights(nc)
    _split_multi_waits(nc)
    return nc


def _prep_inputs(q, k, v, schedules):
    """Per-core input arrays keyed as the programs expect."""
    # flat head g = b*H + h
    qt_nat = q.transpose(0, 2, 3, 1).reshape(B * H, D, S).astype(np.float16)
    kt_nat = k.transpose(0, 2, 3, 1).reshape(B * H, D, S).astype(np.float16)
    kt_nat = kt_nat.reshape(B * H, D, NB, 64)
    # augmented Q^T: rows 64:96 are q-block indicators [qb(q) == j]
    qind = np.zeros((NB, S), np.float16)
    for j in range(NB):
        qind[j, 64 * j : 64 * (j + 1)] = 1.0
    qt_all = np.zeros((B * H, DA, S), np.float16)
    qt_all[:, :D, :] = qt_nat
    qt_all[:, D : D + NB, :] = qind[None]
    # augmented K^T: pair-ordered K rows + mask rows kt[64+j, kb-col] = NEGM
    # where block (qb=j, kb) is dead (block-causal AND sparse mask)
    masks_all = np.asarray(schedules[0]["masks_all"])
    kt_all = np.zeros((B * H, DA, S), np.float16)
    for g in range(B * H):
        order = [kb for p in schedules[g]["pairs"] for kb in p]
        kt_all[g, :D] = kt_nat[g][:, order, :].reshape(D, S)
        m = masks_all[g]  # [32 qb, 32 kb] bool, causal applied separately
        for pos, kb in enumerate(order):
            dead = np.ones(NB, np.float16) * NEGM
            for j in range(NB):
                if j >= kb and m[j, kb]:
                    dead[j] = 0.0
            kt_all[g, D : D + NB, 64 * pos : 64 * (pos + 1)] = dead[:, None]
    v_aug = np.concatenate([v, np.ones((B, S, H, 1), v.dtype)], axis=3)  # [B,S,H,65]
    vb_all = v_aug.transpose(0, 2, 1, 3).reshape(B * H, NB, 64, 65)  # [g, kb, 64, 65]
    # va[g]: per pair t, rows 0:64 = V[kb1] block, rows 64:128 = V[kb2]
    va_all = np.zeros((B * H, 128, 16 * 65), np.float16)
    for g in range(B * H):
        for t, (kb1, kb2) in enumerate(schedules[g]["pairs"]):
            va_all[g, 0:64, 65 * t : 65 * (t + 1)] = vb_all[g, kb1]
            va_all[g, 64:128, 65 * t : 65 * (t + 1)] = vb_all[g, kb2]
    # tri[kl, ql] = 1 where kl <= ql (allowed), both halves
    triu = np.triu(np.ones((64, 64), np.float16))
    tri_full = np.ascontiguousarray(np.concatenate([triu, triu], axis=0))
    # Diagonal-pair pattern [128, 128] for adjacent cols (qb=2t, qb=2t+1):
    # tri on the two diagonal sub-blocks, 1 elsewhere (dead halves are already
    # exact zeros from the QK mask-fold).
    one = np.ones((64, 64), np.float16)
    pat = np.block([[triu, one], [one, triu]]).astype(np.float16)
    pats_full = np.ascontiguousarray(pat)
    in_maps = []
    for c in range(N_CORES):
        sl = slice(HPC * c, HPC * (c + 1))
        in_maps.append(
            {
                "qt": np.ascontiguousarray(qt_all[sl]),
                "kt": np.ascontiguousarray(kt_all[sl]),
                "va": va_all[sl],
                "tri": tri_full,
                "pats": pats_full,
            }
        )
    return in_maps


def _match_pairs(mask, adj_bonus=1.5):
    """Pair k-blocks to maximize overlap of their active-q sets (greedy
    max-weight matching). Overlapping pairs make dual-dense score columns,
    shrinking the union column count that drives QK/exp/PV work. Adjacent
    pairs (i, i+1) get a bonus: their two diagonal-block triangle fixups
    merge into one [128,128] DVE op."""
    act = {
        kb: frozenset(qb for qb in range(kb, NB) if mask[qb, kb]) for kb in range(NB)
    }
    left = set(range(NB))
    pairs = []
    while left:
        best = None
        for i in left:
            for j in left:
                if j <= i:
                    continue
                sc = len(act[i] & act[j]) + (adj_bonus if j == i + 1 else 0.0)
                if best is None or sc > best[0] or (sc == best[0] and (i, j) < best[1:]):
                    best = (sc, i, j)
        _, i, j = best
        pairs.append((i, j))
        left -= {i, j}
    pairs.sort()
    return pairs


def _schedules(block_mask):
    """Per flat head: k-block pairing + column schedule."""
    masks_all = np.asarray(block_mask).reshape(B * H, NB, NB)
    scheds = []
    for g in range(B * H):
        pairs = [(2 * t, 2 * t + 1) for t in range(NB // 2)]
        scheds.append(
            {
                "pairs": pairs,
                "cols": _head_schedule(masks_all[g], pairs, gap=0),
                "masks_all": masks_all,
            }
        )
    return scheds


_PROG_CACHE = {}


def _get_programs(block_mask, schedules):
    key = np.asarray(block_mask).tobytes()
    if key not in _PROG_CACHE:
        _PROG_CACHE[key] = [
            build_program(schedules[HPC * c : HPC * (c + 1)]) for c in range(N_CORES)
        ]
    return _PROG_CACHE[key]


def run_cores(ncs, in_maps, trace=False):
    """Run the 8 per-core programs concurrently on the 8 devices."""
    import jax

    devs = jax.devices()
    results = [None] * N_CORES
    errs = [None] * N_CORES

    def _run(c):
        try:
            with jax.default_device(devs[c]):
                r = run_bass_kernel_spmd(
                    ncs[c], [in_maps[c]], core_ids=[0], trace=trace and c == 0
                )
                results[c] = r
        except Exception as e:  # noqa: BLE001
            errs[c] = e

    threads = [threading.Thread(target=_run, args=(c,)) for c in range(N_CORES)]
    for t in threads:
        t.start()
    for t in threads:
        t.join()
    for c, e in enumerate(errs):
        if e is not None:
            raise RuntimeError(f"core {c} failed") from e
    return results


def kernel(q, k, v, block_mask):
    q = np.asarray(q, dtype=np.float32)
    k = np.asarray(k, dtype=np.float32)
    v = np.asarray(v, dtype=np.float32)
    block_mask = np.asarray(block_mask).astype(bool)

    schedules = _schedules(block_mask)
    in_maps = _prep_inputs(q, k, v, schedules)
    ncs = _get_programs(block_mask, schedules)
    results = run_cores(ncs, in_maps)

    out = np.empty((B, S, H, D), np.float32)
    for c in range(N_CORES):
        ot = results[c].results[0]["ot"]  # [HPC, 65, S]
        for s in range(HPC):
            g = HPC * c + s
            b, h = divmod(g, H)
            o_un = ot[s, :D, :]  # [D, S] unnormalized
            l = ot[s, D, :]  # [S]
            out[b, :, h, :] = (o_un / l[None, :]).T
    return out


# revision 26
# speedup vs baseline: 1.2216x; 1.0114x over previous
"""Block-sparse attention kernel for Trainium2 (8 NeuronCores).

Problem: B=2, S=2048, H=16, Dqk=Dv=64, 64x64 block mask (30% + forced diag),
AND causal. out = softmax(mask(QK^T/8)) @ V.

Strategy
--------
- Shard the 32 (batch, head) pairs across 8 cores, 4 heads per core.
- Each core gets its OWN Bass program with the sparse block schedule baked in
  from its heads' block masks (compiled at call time, run concurrently on the
  8 axon devices).
- Per head, scores are computed TRANSPOSED (S^T[k, q]) so that P^T = exp(S^T)
  lands in SBUF in exactly the layout PV needs (k on partitions) — no on-chip
  transposes anywhere:
    * host supplies Q^T and K^T as [96(d+ind), 2048(s)] fp16, V as [128,
      16*65] fp16 "v-pair" tiles [V[2t]; V[2t+1]] with a ones column (col 64).
    * rows 64:96 of Q^T are 32 q-block indicator rows; rows 64:96 of K^T hold
      -30000 where block (qb=j, kb) is masked off (block-causal AND sparse
      mask): the QK matmul contracts 96 rows and lands masked scores directly,
      so exp() gives exact zeros for dead 64x64 half-blocks — no memsets.
    * k-blocks are processed in pairs (2t, 2t+1) = 128 partitions.
    * QK: matmul(lhsT=K^T pair [96,128], rhs=Q^T qb-run [96,64n]) -> PSUM.
    * exp: one ACT op per 1536-col PSUM chunk (scale=1/8 fused), fp16 out.
    * fixups (DVE): multiply causal triangle into diagonal blocks (the only
      sub-block-granularity mask), one [128,128] pattern op per pair.
    * PV: matmul(lhsT=[V|1] pair [128,65], rhs=P^T run) accumulating O^T[65,
      2048] in PSUM across k-pairs. The program-order-first PV per PSUM bank
      uses start=True (clears the bank's has_written bits; later PVs
      accumulate or overwrite-on-first-touch per element).
    * O^T (unnormalized, with row 64 = softmax denominator l) is copied to
      SBUF and DMA'd out; the host divides and transposes back.
- The per-chunk pipeline is software-pipelined one deep (emit QK(i+1) before
  PV(i)) so the PE never stalls waiting for ACT/DVE of the current chunk.
- A post-scheduling pass drops redundant LDWEIGHTS (bacc emits one per
  matmul; same-weights chains need only the first — weight reloads would
  otherwise dominate PE time for short runs).
- Softmax uses no running max: inputs are N(0,1) so scores/8 stay in a range
  where exp() is safely finite in fp32 (exp(~7) ~ 1e3).
"""

import threading
from contextlib import ExitStack

import numpy as np

import concourse.bass as bass
import concourse.tile as tile
from concourse import mybir
from concourse.bass_utils import run_bass_kernel_spmd
from concourse.vector_clock import ScopedClock

# ----------------------------------------------------------------------------
# Workaround: the installed walrus rejects instructions with more than one
# sync wait. Tile's kernel-tail drain attaches every outstanding clock sem to
# one Drain instruction; split them one wait per Drain.
# ----------------------------------------------------------------------------


def _split_drain_and_barrier(self, tick_clock, wait_clock):
    nc = self.nc
    drain_inst = nc.sync.drain()
    wait_clock.add_sem_waits(
        drain_inst.ins, ScopedClock({None: tick_clock.global_clock})
    )
    si = drain_inst.ins.sync_info
    waits = list(si.on_wait) if si is not None else []
    if len(waits) > 1:
        drain_inst.ins.sync_info = mybir.SyncInfo(
            on_wait=waits[:1], on_update=list(si.on_update)
        )
        for w in waits[1:]:
            d2 = nc.sync.drain()
            d2.ins.sync_info = mybir.SyncInfo(on_wait=[w], on_update=[])
    nc.all_engine_barrier()
    popped = nc._tile_sem_poison_stack.pop()
    assert popped is self._sem_poison
    nc.clear_and_free_semaphores(list(self.sems.allocated().values()))
    nc.all_engine_barrier()


tile.TileContext._drain_and_barrier = _split_drain_and_barrier


def _dedup_ldweights(nc):
    """Bacc lowers every matmul to an explicit Ldweights + non-self-loading
    Matmult pair; short same-weights runs then reload the identical weights
    every matmul, and LDWEIGHTS (~weight_cols/1.2GHz each) dominates PE time.
    Drop an Ldweights when the previously loaded weights are identical,
    preserving its semaphore waits/updates on a PE NOP in its place.

    Runs after TileContext exit (Bacc.compile already moved matmul waits onto
    the Ldweights), BEFORE _split_multi_waits.
    """
    for fn in nc.m.functions:
        for bb in fn.blocks:
            out = []
            last_key = None
            changed = False
            for inst in bb.instructions:
                if isinstance(inst, mybir.InstLdweights):
                    w = inst.ins[0]
                    key = (
                        str(getattr(w, "memref", None)),
                        w.offset,
                        str(w.ap),
                        str(getattr(w, "dtype", None)),
                        inst.tile_position,
                        inst.perf_mode,
                        inst.is_transpose,
                    )
                    if key == last_key:
                        si = inst.sync_info
                        waits = list(si.on_wait) if si else []
                        ups = list(si.on_update) if si else []
                        if waits or ups:
                            out.append(
                                mybir.InstNoOp(
                                    name=nc.get_next_instruction_name(),
                                    engine=inst.engine,
                                    sync_info=mybir.SyncInfo(
                                        on_wait=waits, on_update=ups
                                    ),
                                    bass_nofuse=True,
                                )
                            )
                        changed = True
                        continue
                    last_key = key
                    out.append(inst)
                elif isinstance(inst, mybir.InstMatmult):
                    out.append(inst)  # non-self-loading; weights undisturbed
                elif isinstance(inst, (mybir.InstNoOp, mybir.InstEventSemaphore)):
                    out.append(inst)
                else:
                    if inst.engine == mybir.EngineType.PE:
                        last_key = None
                    out.append(inst)
            if changed:
                bb.instructions = out


def _split_multi_waits(nc):
    """Hoist extra sync waits onto same-engine NOPs (walrus: 1 wait/inst)."""
    for fn in nc.m.functions:
        for bb in fn.blocks:
            out = []
            changed = False
            for inst in bb.instructions:
                si = inst.sync_info
                if si is not None and len(si.on_wait) > 1:
                    waits = list(si.on_wait)
                    for w in waits[:-1]:
                        out.append(
                            mybir.InstNoOp(
                                name=nc.get_next_instruction_name(),
                                engine=inst.engine,
                                sync_info=mybir.SyncInfo(on_wait=[w], on_update=[]),
                                bass_nofuse=True,
                            )
                        )
                    inst.sync_info = mybir.SyncInfo(
                        on_wait=[waits[-1]], on_update=list(si.on_update)
                    )
                    changed = True
                out.append(inst)
            if changed:
                bb.instructions = out

# ---------------------------------------------------------------------------
# Problem constants (hardcoded per the task contract)
# ---------------------------------------------------------------------------
B, S, H, D = 2, 2048, 16, 64
NB = 32  # number of 64-wide blocks along S
N_CORES = 8
HPC = 4  # heads (flat b*H+h) per core
CHUNK = 24  # score col-blocks per PSUM chunk (24*64 = 1536 fp32 = 3 banks)
DA = 96  # augmented contraction: 64 d rows + 32 q-block indicator rows
NEGM = -30000.0  # fp16-safe "minus infinity" for masked score bias
F16 = mybir.dt.float16
F32 = mybir.dt.float32


def _head_schedule(mask, pairs, gap=2):
    """Columns of the S^T score layout for one head.

    mask: [32, 32] bool. Active block (qb, kb) requires qb >= kb (block-level
    causal) and mask[qb, kb]. pairs: 16 (kb1, kb2) k-block pairs; pair t forms
    the 128-partition tile [K[kb1]; K[kb2]].

    Emission order is qb-bank-group-major (g = qb//8), then pair-major, so all
    of an O^T bank's PV writes are contiguous and each (g, t) shares one QK
    weight tile.

    Interior qb-gaps of <= `gap` within a (g, t) sequence are bridged with
    fake columns (top=bot=False -> fully masked, exp gives 0) so QK/PV runs
    merge into fewer, larger matmuls.
    """
    cols = []
    for g in range(NB // 8):
        for t, (kb1, kb2) in enumerate(pairs):
            seq = []
            for qb in range(8 * g, 8 * (g + 1)):
                top = qb >= kb1 and bool(mask[qb, kb1])
                bot = qb >= kb2 and bool(mask[qb, kb2])
                if top or bot:
                    seq.append((qb, top, bot))
            ext = []
            for idx, (qb, top, bot) in enumerate(seq):
                if ext:
                    prev_qb = ext[-1][0]
                    if 1 < qb - prev_qb <= gap + 1:
                        for fqb in range(prev_qb + 1, qb):
                            ext.append((fqb, False, False))
                ext.append((qb, top, bot))
            for qb, top, bot in ext:
                cols.append(
                    {
                        "t": t,
                        "qb": qb,
                        "top": top,
                        "bot": bot,
                        "kb1": kb1,
                        "kb2": kb2,
                        "g": g,
                    }
                )
    return cols


def _is_diag_pair(c, nxt):
    """col c = (qb==kb1, top tri) directly followed by its partner col
    (qb==kb2==qb+1, bot tri) of the same pair -> one [128,128] pattern op."""
    return (
        c["qb"] == c["kb1"]
        and c["top"]
        and nxt is not None
        and nxt["t"] == c["t"]
        and nxt["qb"] == c["qb"] + 1
        and nxt["qb"] == nxt["kb2"]
        and nxt["bot"]
    )


def _runs(chunk, key_consecutive, bank_of, flags=None):
    """Split a chunk (list of (idx, col)) into affine matmul runs.

    key_consecutive(prev, cur) -> bool: can cur extend the run?
    bank_of(idx, col) -> int: PSUM bank id of the run target; run must stay in
      one bank.
    flags(col) -> hashable: must be uniform within a run (or None).
    """
    runs = []
    cur = []
    for item in chunk:
        if cur:
            _, pc = cur[-1]
            _, cc = item
            ok = (
                key_consecutive(pc, cc)
                and bank_of(*item) == bank_of(*cur[0])
                and (flags is None or flags(cc) == flags(pc))
            )
            if ok:
                cur.append(item)
                continue
            runs.append(cur)
        cur = [item]
    if cur:
        runs.append(cur)
    return runs


def _chunks_of(cols):
    """Cut cols into chunks of <= CHUNK, never splitting a diagonal pair."""
    chunks = []
    cur = []
    i = 0
    while i < len(cols):
        nxt = cols[i + 1] if i + 1 < len(cols) else None
        take = 2 if _is_diag_pair(cols[i], nxt) else 1
        if len(cur) + take > CHUNK:
            chunks.append(cur)
            cur = []
        cur.extend(cols[i : i + take])
        i += take
    if cur:
        chunks.append(cur)
    return chunks


def build_program(schedules):
    """Build the Bass program for one core.

    schedules: list of HPC dicts {"pairs": [(kb1, kb2)]*16, "cols": [...]}.
    """
    nc = bass.Bass()
    qt = nc.declare_dram_parameter("qt", [HPC, DA, S], F16, isOutput=False)
    kt = nc.declare_dram_parameter("kt", [HPC, DA, S], F16, isOutput=False)
    va = nc.declare_dram_parameter("va", [HPC, 128, 16 * 65], F16, isOutput=False)
    tri = nc.declare_dram_parameter("tri", [128, 64], F16, isOutput=False)
    pats = nc.declare_dram_parameter("pats", [128, 128], F16, isOutput=False)
    ot = nc.declare_dram_parameter("ot", [HPC, 65, S], F32, isOutput=True)

    with tile.TileContext(nc) as tc, ExitStack() as ctx:
        const = ctx.enter_context(tc.tile_pool(name="const", bufs=1))
        pts = ctx.enter_context(tc.tile_pool(name="pts", bufs=3))
        outp = ctx.enter_context(tc.tile_pool(name="outp", bufs=3))
        psS = ctx.enter_context(tc.tile_pool(name="psS", bufs=2, space="PSUM"))
        psO = ctx.enter_context(tc.tile_pool(name="psO", bufs=2, space="PSUM"))

        # All input DMAs share the sync hardware queue (other engines' queues
        # would park a completion-wait in front of their compute work).
        # Transfer order = need order: the tiny fixup constants, then head 0's
        # tensors, then the rest; the first fixup/PV stall otherwise.
        tri_t = const.tile([128, 64], F16, tag="tri")
        nc.sync.dma_start(out=tri_t[:], in_=tri[:])
        pats_t = const.tile([128, 128], F16, tag="pats")
        nc.sync.dma_start(out=pats_t[:], in_=pats[:])
        qts, kts, vas = [], [], []
        for s in range(HPC):
            qs = const.tile([DA, S], F16, tag=f"qt{s}")
            ks = const.tile([DA, S], F16, tag=f"kt{s}")
            vs = const.tile([128, 16 * 65], F16, tag=f"va{s}")
            if s == 0:
                # Head 0 paces the pipeline start: land the first-half columns
                # (pairs t<8 / qb-groups 0-1 — all chunk-0 ever touches)
                # before the rest so the first QK issues ~3us earlier.
                nc.sync.dma_start(out=ks[:, 0 : S // 2], in_=kt[s][:, 0 : S // 2])
                nc.sync.dma_start(out=qs[:, 0 : S // 2], in_=qt[s][:, 0 : S // 2])
                nc.sync.dma_start(out=vs[:], in_=va[s])
                nc.sync.dma_start(out=ks[:, S // 2 :], in_=kt[s][:, S // 2 :])
                nc.sync.dma_start(out=qs[:, S // 2 :], in_=qt[s][:, S // 2 :])
            else:
                nc.sync.dma_start(out=ks[:], in_=kt[s])
                nc.sync.dma_start(out=qs[:], in_=qt[s])
                nc.sync.dma_start(out=vs[:], in_=va[s])
            qts.append(qs)
            kts.append(ks)
            vas.append(vs)
        zeros = const.tile([128, 512], F16, tag="zeros")
        nc.vector.memset(zeros[:], 0.0)

        # PE warm-up: the HAM clock gate keeps a cold PE at 1.2 GHz; burn
        # ~4 us of dummy matmuls (overlapping the input DMAs) to reach 2.4.
        wps = psS.tile([128, 64 * CHUNK], F32, tag="ps")
        for _ in range(8):
            nc.tensor.matmul(
                wps[:, 0:512],
                lhsT=zeros[:, 0:128],
                rhs=zeros[:, 0:512],
                start=True,
                stop=True,
            )

        # One flat chunk stream across all heads so the software pipeline
        # (and the PE) never drains at head boundaries.
        stream = []  # (s, key=(s, ci), chunk_cols)
        last_chunk_of_group = {}  # (s, g) -> key of chunk with g's last col
        for s in range(HPC):
            for ci, chunk_cols in enumerate(_chunks_of(schedules[s]["cols"])):
                key = (s, ci)
                stream.append((s, key, chunk_cols))
                for col in chunk_cols:
                    last_chunk_of_group[(s, col["qb"] // 8)] = key

        oTs = {}  # (s, g) -> [tile, opened_flag]

        def get_oT(s_, g_):
            if (s_, g_) not in oTs:
                oTs[(s_, g_)] = [
                    psO.tile([128, 512], F32, name=f"oT{s_}_{g_}", tag="psO"),
                    False,
                ]
            return oTs[(s_, g_)]

        def close_group(s_, g_):
            t_, _ = oTs.pop((s_, g_))
            o_sb = outp.tile([65, 512], F32, name=f"osb{s_}_{g_}", tag="o")
            nc.vector.tensor_copy(out=o_sb[:], in_=t_[0:65, :])
            nc.sync.dma_start(
                out=ot[s_][:, 512 * g_ : 512 * (g_ + 1)], in_=o_sb[:]
            )

        def emit_qk(s_, chunk, ps):
            qk = _runs(
                chunk,
                key_consecutive=lambda p, c: p["t"] == c["t"]
                and c["qb"] == p["qb"] + 1,
                bank_of=lambda i, c: i // 8,
            )
            for run in qk:
                i0, rc = run[0]
                n = len(run)
                nc.tensor.matmul(
                    ps[:, 64 * i0 : 64 * (i0 + n)],
                    lhsT=kts[s_][:, 128 * rc["t"] : 128 * (rc["t"] + 1)],
                    rhs=qts[s_][:, 64 * rc["qb"] : 64 * (rc["qb"] + n)],
                    start=True,
                    stop=True,
                )

        def emit_fixups(chunk, pt):
            # Only sub-block mask left after the QK mask-fold: the causal
            # triangle on diagonal blocks. Adjacent diag pairs use one
            # [128,128] pattern op; stragglers use a [64,64] tri op.
            L = len(chunk)
            i = 0
            while i < L:
                c = chunk[i][1]
                if _is_diag_pair(c, chunk[i + 1][1] if i + 1 < L else None):
                    nc.vector.tensor_mul(
                        pt[:, 64 * i : 64 * (i + 2)],
                        pt[:, 64 * i : 64 * (i + 2)],
                        pats_t[:],
                    )
                    i += 2
                    continue
                if c["top"] and c["qb"] == c["kb1"]:
                    nc.vector.tensor_mul(
                        pt[0:64, 64 * i : 64 * (i + 1)],
                        pt[0:64, 64 * i : 64 * (i + 1)],
                        tri_t[0:64],
                    )
                if c["bot"] and c["qb"] == c["kb2"]:
                    nc.vector.tensor_mul(
                        pt[64:128, 64 * i : 64 * (i + 1)],
                        pt[64:128, 64 * i : 64 * (i + 1)],
                        tri_t[64:128],
                    )
                i += 1

        def emit_pv(s_, chunk, pt):
            pv = _runs(
                chunk,
                key_consecutive=lambda p, c: p["t"] == c["t"]
                and c["qb"] == p["qb"] + 1,
                bank_of=lambda i, c: c["qb"] // 8,
            )
            for run in pv:
                i0, rc = run[0]
                n = len(run)
                g_ = rc["qb"] // 8
                ent = get_oT(s_, g_)
                first = not ent[1]
                ent[1] = True
                q0 = rc["qb"] - 8 * g_
                nc.tensor.matmul(
                    ent[0][0:65, 64 * q0 : 64 * (q0 + n)],
                    lhsT=vas[s_][:, 65 * rc["t"] : 65 * (rc["t"] + 1)],
                    rhs=pt[:, 64 * i0 : 64 * (i0 + n)],
                    start=first,
                    stop=True,
                    skip_group_check=True,
                )

        # Software-pipelined chunk loop: PE order is QK(0), QK(1), PV(0),
        # QK(2), PV(1), ..., PV(last) so the PE works on the next chunk's
        # scores while ACT+DVE process the current one.
        pending = None  # (s, key, chunk, pt) awaiting PV emission
        for s, key, chunk_cols in stream:
            chunk = list(enumerate(chunk_cols))
            L = len(chunk)
            ps = psS.tile([128, 64 * CHUNK], F32, tag="ps")
            emit_qk(s, chunk, ps)
            if pending is not None:
                emit_pv(pending[0], pending[2], pending[3])
                for sg in [
                    sg2
                    for sg2, lc in last_chunk_of_group.items()
                    if lc == pending[1]
                ]:
                    close_group(*sg)
            pt = pts.tile([128, 64 * CHUNK], F16, tag="pt")
            nc.scalar.activation(
                out=pt[:, : 64 * L],
                in_=ps[:, : 64 * L],
                func=mybir.ActivationFunctionType.Exp,
                scale=0.125,
            )
            emit_fixups(chunk, pt)
            pending = (s, key, chunk, pt)
        emit_pv(pending[0], pending[2], pending[3])
        for sg in sorted(oTs):
            close_group(*sg)

    _dedup_ldweights(nc)
    _split_multi_waits(nc)
    return nc


def _prep_inputs(q, k, v, schedules):
    """Per-core input arrays keyed as the programs expect."""
    # flat head g = b*H + h
    qt_nat = q.transpose(0, 2, 3, 1).reshape(B * H, D, S).astype(np.float16)
    kt_nat = k.transpose(0, 2, 3, 1).reshape(B * H, D, S).astype(np.float16)
    kt_nat = kt_nat.reshape(B * H, D, NB, 64)
    # augmented Q^T: rows 64:96 are q-block indicators [qb(q) == j]
    qind = np.zeros((NB, S), np.float16)
    for j in range(NB):
        qind[j, 64 * j : 64 * (j + 1)] = 1.0
    qt_all = np.zeros((B * H, DA, S), np.float16)
    qt_all[:, :D, :] = qt_nat
    qt_all[:, D : D + NB, :] = qind[None]
    # augmented K^T: pair-ordered K rows + mask rows kt[64+j, kb-col] = NEGM
    # where block (qb=j, kb) is dead (block-causal AND sparse mask)
    masks_all = np.asarray(schedules[0]["masks_all"])
    kt_all = np.zeros((B * H, DA, S), np.float16)
    for g in range(B * H):
        order = [kb for p in schedules[g]["pairs"] for kb in p]
        kt_all[g, :D] = kt_nat[g][:, order, :].reshape(D, S)
        m = masks_all[g]  # [32 qb, 32 kb] bool, causal applied separately
        for pos, kb in enumerate(order):
            dead = np.ones(NB, np.float16) * NEGM
            for j in range(NB):
                if j >= kb and m[j, kb]:
                    dead[j] = 0.0
            kt_all[g, D : D + NB, 64 * pos : 64 * (pos + 1)] = dead[:, None]
    v_aug = np.concatenate([v, np.ones((B, S, H, 1), v.dtype)], axis=3)  # [B,S,H,65]
    vb_all = v_aug.transpose(0, 2, 1, 3).reshape(B * H, NB, 64, 65)  # [g, kb, 64, 65]
    # va[g]: per pair t, rows 0:64 = V[kb1] block, rows 64:128 = V[kb2]
    va_all = np.zeros((B * H, 128, 16 * 65), np.float16)
    for g in range(B * H):
        for t, (kb1, kb2) in enumerate(schedules[g]["pairs"]):
            va_all[g, 0:64, 65 * t : 65 * (t + 1)] = vb_all[g, kb1]
            va_all[g, 64:128, 65 * t : 65 * (t + 1)] = vb_all[g, kb2]
    # tri[kl, ql] = 1 where kl <= ql (allowed), both halves
    triu = np.triu(np.ones((64, 64), np.float16))
    tri_full = np.ascontiguousarray(np.concatenate([triu, triu], axis=0))
    # Diagonal-pair pattern [128, 128] for adjacent cols (qb=2t, qb=2t+1):
    # tri on the two diagonal sub-blocks, 1 elsewhere (dead halves are already
    # exact zeros from the QK mask-fold).
    one = np.ones((64, 64), np.float16)
    pat = np.block([[triu, one], [one, triu]]).astype(np.float16)
    pats_full = np.ascontiguousarray(pat)
    in_maps = []
    for c in range(N_CORES):
        sl = slice(HPC * c, HPC * (c + 1))
        in_maps.append(
            {
                "qt": np.ascontiguousarray(qt_all[sl]),
                "kt": np.ascontiguousarray(kt_all[sl]),
                "va": va_all[sl],
                "tri": tri_full,
                "pats": pats_full,
            }
        )
    return in_maps


def _match_pairs(mask, adj_bonus=1.5):
    """Pair k-blocks to maximize overlap of their active-q sets (greedy
    max-weight matching). Overlapping pairs make dual-dense score columns,
    shrinking the union column count that drives QK/exp/PV work. Adjacent
    pairs (i, i+1) get a bonus: their two diagonal-block triangle fixups
    merge into one [128,128] DVE op."""
    act = {
        kb: frozenset(qb for qb in range(kb, NB) if mask[qb, kb]) for kb in range(NB)
    }
    left = set(range(NB))
    pairs = []
    while left:
        best = None
        for i in left:
            for j in left:
                if j <= i:
                    continue
                sc = len(act[i] & act[j]) + (adj_bonus if j == i + 1 else 0.0)
                if best is None or sc > best[0] or (sc == best[0] and (i, j) < best[1:]):
                    best = (sc, i, j)
        _, i, j = best
        pairs.append((i, j))
        left -= {i, j}
    pairs.sort()
    return pairs


def _schedules(block_mask):
    """Per flat head: k-block pairing + column schedule."""
    masks_all = np.asarray(block_mask).reshape(B * H, NB, NB)
    scheds = []
    for g in range(B * H):
        pairs = [(2 * t, 2 * t + 1) for t in range(NB // 2)]
        scheds.append(
            {
                "pairs": pairs,
                "cols": _head_schedule(masks_all[g], pairs, gap=0),
                "masks_all": masks_all,
            }
        )
    return scheds


_PROG_CACHE = {}


def _get_programs(block_mask, schedules):
    key = np.asarray(block_mask).tobytes()
    if key not in _PROG_CACHE:
        _PROG_CACHE[key] = [
            build_program(schedules[HPC * c : HPC * (c + 1)]) for c in range(N_CORES)
        ]
    return _PROG_CACHE[key]


def run_cores(ncs, in_maps, trace=False):
    """Run the 8 per-core programs concurrently on the 8 devices."""
    import jax

    devs = jax.devices()
    results = [None] * N_CORES
    errs = [None] * N_CORES

    def _run(c):
        try:
            with jax.default_device(devs[c]):
                r = run_bass_kernel_spmd(
                    ncs[c], [in_maps[c]], core_ids=[0], trace=trace and c == 0
                )
                results[c] = r
        except Exception as e:  # noqa: BLE001
            errs[c] = e

    threads = [threading.Thread(target=_run, args=(c,)) for c in range(N_CORES)]
    for t in threads:
        t.start()
    for t in threads:
        t.join()
    for c, e in enumerate(errs):
        if e is not None:
            raise RuntimeError(f"core {c} failed") from e
    return results


def kernel(q, k, v, block_mask):
    q = np.asarray(q, dtype=np.float32)
    k = np.asarray(k, dtype=np.float32)
    v = np.asarray(v, dtype=np.float32)
    block_mask = np.asarray(block_mask).astype(bool)

    schedules = _schedules(block_mask)
    in_maps = _prep_inputs(q, k, v, schedules)
    ncs = _get_programs(block_mask, schedules)
    results = run_cores(ncs, in_maps)

    out = np.empty((B, S, H, D), np.float32)
    for c in range(N_CORES):
        ot = results[c].results[0]["ot"]  # [HPC, 65, S]
        for s in range(HPC):
            g = HPC * c + s
            b, h = divmod(g, H)
            o_un = ot[s, :D, :]  # [D, S] unnormalized
            l = ot[s, D, :]  # [S]
            out[b, :, h, :] = (o_un / l[None, :]).T
    return out
